# revision 5
# baseline (speedup 1.0000x reference)
"""AlleleEmbedding Trainium2 kernel (8 NeuronCores, SPMD).

out[b,p,:] = (sum_ploidy allele_table[alleles[b,p,:]]) @ kernel_table[positions[b,p]].reshape(D,D)
             + bias_table[positions[b,p]]

Strategy: shard kernel_table/bias_table over positions (2500 rows per core).
The host routes each (b,p) pair to the core owning its position row, pads the
per-core pair lists to a common length, and the device does:
  - indirect-DMA gather of the per-pair [D,D] weight row -> one SBUF partition per pair
  - allele embedding a = counts @ allele_table via TensorE (counts built host-side
    from the int32 allele indices)
  - P = G * broadcast(a)  on VectorE, segmented reduce over d -> [128, D]
  - + gathered bias, DMA out
Host scatters per-core outputs back into the full [B,P,D] tensor.
"""

import os
import numpy as np

B, P, PLOIDY = 8, 5000, 2
NALLELES, NPOS, D = 16, 20000, 64
NCORES = 8
RPC = NPOS // NCORES  # rows per core: 2500

LAST_EXEC_TIME_NS = None

_NC_CACHE = {}


def _build_nc(nmax: int):
    import concourse.bass as bass
    import concourse.bacc as bacc
    import concourse.tile as tile
    from concourse import mybir

    f32 = mybir.dt.float32
    nc = bacc.Bacc(None, target_bir_lowering=False, debug=False)
    kt = nc.declare_dram_parameter("kt", [RPC, D * D], f32, isOutput=False)
    bt = nc.declare_dram_parameter("bt", [RPC, D], f32, isOutput=False)
    at = nc.declare_dram_parameter("at", [NALLELES, D], f32, isOutput=False)
    ct = nc.declare_dram_parameter("ct", [NALLELES, nmax], f32, isOutput=False)
    idx = nc.declare_dram_parameter("idx", [nmax, 1], mybir.dt.int32, isOutput=False)
    out = nc.declare_dram_parameter("out", [nmax, D], f32, isOutput=True)

    nchunks = nmax // 128
    with tile.TileContext(nc) as tc:
        with (
            tc.tile_pool(name="const", bufs=1) as constp,
            tc.tile_pool(name="g", bufs=3) as gp,
            tc.tile_pool(name="small", bufs=4) as sp,
            tc.tile_pool(name="ps", bufs=2, space="PSUM") as pp,
        ):
            at_t = constp.tile([NALLELES, D], f32)
            nc.sync.dma_start(out=at_t[:], in_=at[:])
            ct_t = constp.tile([NALLELES, nmax], f32)
            nc.sync.dma_start(out=ct_t[:], in_=ct[:])
            tc.strict_bb_all_engine_barrier()
            for c in range(nchunks):
                s = c * 128
                idx_t = sp.tile([128, 1], mybir.dt.int32, tag="idx")
                nc.sync.dma_start(out=idx_t[:], in_=idx[s : s + 128, :])
                g_t = gp.tile([128, D * D], f32, tag="g")
                nc.gpsimd.indirect_dma_start(
                    out=g_t[:],
                    out_offset=None,
                    in_=kt[:],
                    in_offset=bass.IndirectOffsetOnAxis(ap=idx_t[:, :1], axis=0),
                )
                b_t = sp.tile([128, D], f32, tag="b")
                nc.gpsimd.indirect_dma_start(
                    out=b_t[:],
                    out_offset=None,
                    in_=bt[:],
                    in_offset=bass.IndirectOffsetOnAxis(ap=idx_t[:, :1], axis=0),
                )
                a_ps = pp.tile([128, D], f32, tag="aps")
                nc.tensor.matmul(
                    out=a_ps[:],
                    lhsT=ct_t[:, s : s + 128],
                    rhs=at_t[:],
                    start=True,
                    stop=True,
                )
                a_sb = sp.tile([128, D], f32, tag="asb")
                nc.scalar.copy(out=a_sb[:], in_=a_ps[:])
                # P[p, d, e] = G[p, d, e] * a[p, d]
                g3 = g_t[:].rearrange("p (d e) -> p d e", d=D)
                a3 = a_sb[:].unsqueeze(2).to_broadcast([128, D, D])
                nc.vector.tensor_tensor(
                    out=g3, in0=g3, in1=a3, op=mybir.AluOpType.mult
                )
                # r[p, e] = sum_d P[p, d, e]
                pv = g_t[:].rearrange("p (d e) -> p e d", d=D)
                r_t = sp.tile([128, D], f32, tag="r")
                nc.vector.tensor_reduce(
                    out=r_t[:], in_=pv, axis=mybir.AxisListType.X, op=mybir.AluOpType.add
                )
                nc.vector.tensor_tensor(
                    out=r_t[:], in0=r_t[:], in1=b_t[:], op=mybir.AluOpType.add
                )
                nc.sync.dma_start(out=out[s : s + 128, :], in_=r_t[:])
    nc.finalize()
    return nc


def kernel(alleles, positions, allele_table, kernel_table, bias_table):
    global LAST_EXEC_TIME_NS
    from concourse.bass_utils import run_bass_kernel_spmd

    alleles = np.asarray(alleles)
    positions = np.asarray(positions)
    allele_table = np.ascontiguousarray(np.asarray(allele_table), dtype=np.float32)
    kernel_table = np.ascontiguousarray(np.asarray(kernel_table), dtype=np.float32)
    bias_table = np.ascontiguousarray(np.asarray(bias_table), dtype=np.float32)

    pos = positions.reshape(-1).astype(np.int64)  # [B*P]
    al = alleles.reshape(-1, PLOIDY)  # [B*P, 2]
    npairs = pos.shape[0]

    owner = pos // RPC
    local_row = pos % RPC
    order = np.lexsort((local_row, owner))  # by core, then by row within core
    counts_per_core = np.bincount(owner, minlength=NCORES)
    nmax = int(np.ceil(max(counts_per_core.max(), 128) / 128) * 128)

    # one-hot-sum counts over ploidy: [B*P, 16]
    cnt = (al[:, :, None] == np.arange(NALLELES)[None, None, :]).sum(1)
    cnt = cnt.astype(np.float32)

    if nmax not in _NC_CACHE:
        _NC_CACHE[nmax] = _build_nc(nmax)
    nc = _NC_CACHE[nmax]

    in_maps = []
    core_slices = []
    start = 0
    for c in range(NCORES):
        n_c = int(counts_per_core[c])
        sel = order[start : start + n_c]
        core_slices.append(sel)
        start += n_c
        idx_c = np.zeros((nmax, 1), dtype=np.int32)
        idx_c[:n_c, 0] = local_row[sel]
        ct_c = np.zeros((NALLELES, nmax), dtype=np.float32)
        ct_c[:, :n_c] = cnt[sel].T
        in_maps.append(
            {
                "kt": kernel_table[c * RPC : (c + 1) * RPC],
                "bt": bias_table[c * RPC : (c + 1) * RPC],
                "at": allele_table,
                "ct": ct_c,
                "idx": idx_c,
            }
        )

    trace = bool(int(os.environ.get("BASS_KERNEL_TRACE", "0")))
    res = run_bass_kernel_spmd(nc, in_maps, core_ids=list(range(NCORES)), trace=trace)
    LAST_EXEC_TIME_NS = res.exec_time_ns

    out_full = np.zeros((npairs, D), dtype=np.float32)
    for c in range(NCORES):
        sel = core_slices[c]
        out_c = np.asarray(res.results[c]["out"])
        out_full[sel] = out_c[: len(sel)]
    return out_full.reshape(B, P, D)
